# revision 16
# baseline (speedup 1.0000x reference)
"""CenterLoss Trainium2 kernel — raw Bacc, chunk-pipelined bf16 (v8).

Per core (512 samples, 4 chunks of 128), bf16 payloads (rel tol is
2e-2; bf16 noise on 2M summed squared terms averages out to ~1e-5):

  sync(SP)   : idx + x DMAs (hoisted into entry, HWDGE); out DMA of
               d [128, 4] f32. All DMAs stay on SP: putting any DMA on
               the ACT HWDGE queue grows Scalar's end-of-program
               semaphore-reset chain from 9 to 51 sems (~4.6us serial)
  scalar(ACT): warm-up Square (hoists act-table load); per-chunk
               Square+accum -> d[:, n]
  vector     : per-chunk tensor_sub, pipelined under the serial
               ~1.1us/chunk gather descriptor generations
  gpsimd     : 4x indirect gather on the single SWDGE queue (the ucode
               only supports [128, 1] offset tables; multiple queues
               neither parallelize descriptor-gen nor stay correct;
               dma_gather/InstDMAGatherAnt is a Q7 software path and
               ~15us slower)
Layouts (p = partition, n = chunk):
  lab_t[p, n]  = labels[p*4 + n]
  x_t[p, n, :] = x[p*4 + n, :]      (plain reshape)
  c_t[p, n, :] = centers[lab_t[p, n]]
  d[p, n] = ||x_t[p,n,:] - c_t[p,n,:]||^2   (ACT Square accumulator)
partial = d[128, 4] per core; host sums across partitions and cores.
"""

import sys

import numpy as np

if "/opt/trn_rl_repo" not in sys.path:
    sys.path.insert(0, "/opt/trn_rl_repo")

import ml_dtypes

B = 4096
D = 256
C = 8192
M = 8
SHARD = B // M   # 512
P = 128
NT = SHARD // P  # 4

_CACHE = {}


def build_nc():
    import concourse.bacc as bacc
    import concourse.bass as bass
    import concourse.mybir as mybir

    f32 = mybir.dt.float32
    bf16 = mybir.dt.bfloat16
    i32 = mybir.dt.int32

    nc = bacc.Bacc("TRN2")
    x = nc.dram_tensor("x", [P, NT, D], bf16, kind="ExternalInput")
    lab = nc.dram_tensor("lab", [P, NT], i32, kind="ExternalInput")
    cen = nc.dram_tensor("cen", [C, D], bf16, kind="ExternalInput")
    out = nc.dram_tensor("out", [P, NT], f32, kind="ExternalOutput")

    ones_bf = nc.const_aps.aps[(bf16, 1.0)]  # [128, 1] const, preamble memset

    with (
        nc.sbuf_tensor("x_t", [P, NT, D], bf16) as x_t,
        nc.sbuf_tensor("c_t", [P, NT, D], bf16) as c_t,
        nc.sbuf_tensor("diff", [P, NT, D], bf16) as diff,
        nc.sbuf_tensor("sq", [P, NT, D], bf16) as sq,
        nc.sbuf_tensor("lab_t", [P, NT], i32) as lab_t,
        nc.sbuf_tensor("d", [P, NT], f32) as d,
        nc.sbuf_tensor("warm", [1, 1], bf16) as warm,
        nc.semaphore("i_s") as i_s,
        nc.semaphore("x_s") as x_s,
        nc.semaphore("g0_s") as g0_s,
        nc.semaphore("g1_s") as g1_s,
        nc.semaphore("g2_s") as g2_s,
        nc.semaphore("g3_s") as g3_s,
        nc.semaphore("v_s") as v_s,
        nc.semaphore("a_s") as a_s,
        nc.semaphore("o_s") as o_s,
        nc.Block() as block,
    ):
        g_sems = (g0_s, g1_s, g2_s, g3_s)
        hoist = []

        @block.sync
        def _(sync):
            hoist.append(
                ("SP", sync.dma_start(lab_t[:, :], lab[:, :]).then_inc(i_s, 16))
            )
            hoist.append(
                ("SP", sync.dma_start(x_t[:, :, :], x[:, :, :]).then_inc(x_s, 16))
            )
            sync.wait_ge(a_s, NT)
            hoist.append(
                (None, sync.dma_start(out[:, :], d[:, :]).then_inc(o_s, 16))
            )

        @block.gpsimd
        def _(g):
            g.wait_ge(i_s, 16)
            for n, gs in enumerate(g_sems):
                g.indirect_dma_start(
                    out=c_t[:, n, :],
                    out_offset=None,
                    in_=cen[:, :],
                    in_offset=bass.IndirectOffsetOnAxis(
                        ap=lab_t[:, n : n + 1], axis=0
                    ),
                ).then_inc(gs, 16)

        @block.vector
        def _(v):
            v.wait_ge(x_s, 16)
            for n, gs in enumerate(g_sems):
                v.wait_ge(gs, 16)
                v.tensor_sub(
                    diff[:, n, :], x_t[:, n, :], c_t[:, n, :]
                ).then_inc(v_s, 1)

        @block.scalar
        def _(s):
            # dummy op forces the Square act-table load at ACT program
            # start, off the critical path
            s.activation(
                warm[:, :], ones_bf[:1, :], mybir.ActivationFunctionType.Square
            )
            for n in range(NT):
                s.wait_ge(v_s, n + 1)
                h = s.activation(
                    sq[:, n, :],
                    diff[:, n, :],
                    mybir.ActivationFunctionType.Square,
                    accum_out=d[:, n : n + 1],
                )
                h.then_inc(a_s, 1)

    # Hoist the input DMAs into the entry block, after each engine's
    # barrier-arrival DRAIN but before its release EVSEM
    # ("barrier_<Eng>_*"): the DMA then issues during the const-init
    # barrier window and its ~2.4us completion chain overlaps it.
    entry = nc.m.functions[0].blocks[0]
    for eng_name, handle in hoist:
        if eng_name is None:
            continue
        inst = handle.ins
        for blk in nc.m.functions[0].blocks:
            if inst in blk.instructions:
                blk.instructions.remove(inst)
                break
        barrier_idx = next(
            i
            for i, ins in enumerate(entry.instructions)
            if ins.name.startswith(f"barrier_{eng_name}")
        )
        entry.instructions.insert(barrier_idx, inst)

    # End-block restructure for SP: its DRAIN carries the
    # barrier-arrival inc. Move the arrival inc to a fresh EVSEM placed
    # before the out-DMA issue and run the drain after the barrier
    # passes, so the other engines' teardown overlaps the out DMA.
    end_blk = nc.m.functions[0].blocks[-1]
    act_drain = next(
        ins
        for ins in end_blk.instructions
        if isinstance(ins, mybir.InstDrain)
        and ins.engine == mybir.EngineType.SP
    )
    act_evsem = next(
        ins
        for ins in end_blk.instructions
        if ins.name.startswith("barrier_SP")
    )
    arrive = mybir.InstEventSemaphore(
        name=nc.get_next_instruction_name(), ins=[], outs=[]
    )
    arrive.engine = mybir.EngineType.SP
    arrive.sync_info = act_drain.sync_info
    act_drain.sync_info = None
    nc.register_instruction(arrive)
    end_blk.instructions.remove(act_drain)
    ei = end_blk.instructions.index(act_evsem)
    end_blk.instructions.insert(ei + 1, act_drain)

    # Place the barrier arrival just before the out-DMA issue so the
    # other engines' teardown overlaps the out DMA. (Issuing the out
    # DMA before the final ACC_READ was tried and races on hardware:
    # cold runs read d[:, 3] before it lands.)
    out_inst = hoist[-1][1].ins
    body_blk = next(
        blk
        for blk in nc.m.functions[0].blocks
        if out_inst in blk.instructions
    )
    oi = body_blk.instructions.index(out_inst)
    body_blk.instructions.insert(oi, arrive)

    nc.compile()
    return nc


def _get_nc():
    if "nc" not in _CACHE:
        _CACHE["nc"] = build_nc()
    return _CACHE["nc"]


def make_in_maps(x, labels, centers):
    bf16 = ml_dtypes.bfloat16
    x = np.ascontiguousarray(np.asarray(x), dtype=np.float32).astype(bf16)
    labels = np.ascontiguousarray(np.asarray(labels)).astype(np.int32)
    centers = np.ascontiguousarray(
        np.asarray(centers), dtype=np.float32
    ).astype(bf16)
    in_maps = []
    for i in range(M):
        sl = slice(i * SHARD, (i + 1) * SHARD)
        in_maps.append(
            {
                # x_t[p, n, :] = x[p*NT + n, :] — plain reshape
                "x": np.ascontiguousarray(x[sl].reshape(P, NT, D)),
                "lab": np.ascontiguousarray(labels[sl].reshape(P, NT)),
                "cen": centers,
            }
        )
    return in_maps


def finish(partials):
    total = float(np.sum(np.asarray(partials, dtype=np.float64)))
    total += B * (C - 1) * 1e-12  # masked-out entries clamp to 1e-12
    return np.float32(total / B)


def kernel(x, labels, centers):
    from concourse import bass_utils

    nc = _get_nc()
    res = bass_utils.run_bass_kernel_spmd(
        nc, make_in_maps(x, labels, centers), list(range(M))
    )
    return finish([np.asarray(r["out"], dtype=np.float64) for r in res.results])
